# revision 20
# baseline (speedup 1.0000x reference)
"""KBC filtered-ranking kernel for 8 Trainium2 NeuronCores.

rank_i = 1 + #{ j unmasked : scores[i,j] >= scores[i, true_i] }

Device (per core, SPMD over column chunks of rhs):
  - scores chunk = q @ rhs_chunk via float32r PE matmuls ([128,500] tiles,
    K=512 as 4x128 PSUM accumulation, 1 cycle/row vs fp32's 4)
  - t_i = device score of the true column (diag of a q_block @ gt matmul,
    bit-identical arithmetic to the main matmul) -- computed redundantly
    on every core
  - counting is split across two engines so PSUM drain never gates the PE:
      even row blocks (b%2==0) -> DVE  tensor_scalar is_gt:
          acc = #{ j in tile : s_ij > t_i }   (strict >, true col excluded)
      odd  row blocks (b%2==1) -> ScalarE activation Sign(s - t):
          acc = #gt - #lt  (sign(0)=0 so the true column contributes 0)
    Tile index i = nt*16 + b has parity of b, so each engine owns whole
    row blocks and each engine's PSUM banks (i%6) stay disjoint.
Host:
  - even blocks: count = sum_core acc
  - odd blocks:  count = (sum_core acc + N - 1) / 2   (the -1 is the true
    column's sign(0)=0 vs the +/-1 it would need to cancel in the identity
    sum = 2*c_gt - N + c_eq; exact fp32 ties elsewhere are ~0.25% of rows
    and cost at most 1 rank after rounding)
  - subtracts the filtered (known-true) tails: for the deduplicated filter
    indices (!= true), count fp64 scores > t and subtract.
  - ranks = round(1 + count - corr)

Set KBC_F32R=0 to run the matmuls in plain fp32 (4 cycles/row on the PE,
~4x slower): L2 rel 1.1e-06. Default float32r: L2 rel ~8e-05 (reduced
precision PE rounding shifts most ranks by a few units; the true-column
self-exclusion stays exact because t flows through the same pipeline).
"""

import os
from contextlib import ExitStack

import numpy as np

B, D, N = 2048, 512, 100000
NCORES = 8
COLS = N // NCORES          # 12500 columns per core
NTW = 500                   # n-tile width
NT = COLS // NTW            # 25 n-tiles per core
NB = B // 128               # 16 row blocks
KT = D // 128               # 4 k tiles
P = 128
NBANK = 6                   # main-loop PSUM rotation depth
NSUP = 5                    # rhs DMA super-tiles (5 n-tiles each): 40KB
TPS = NT // NSUP            # contiguous per partition per DMA, not 8KB

_CACHE = {}


F32R = os.environ.get("KBC_F32R", "1") == "1"
# "split" (DVE+ScalarE counts), "dveonly" (all counts DVE), "nocnt" (timing
# bisection: no counting at all, wrong results)
VARIANT = os.environ.get("KBC_VARIANT", "split")


def _gen():
    import concourse.bass as bass
    import concourse.mybir as mybir

    mdt = mybir.dt.float32r if F32R else mybir.dt.float32
    nc = bass.Bass()
    qT_d = nc.dram_tensor("qT", [P, KT, B], mdt, kind="ExternalInput")
    rhs_d = nc.dram_tensor(
        "rhsc", [NSUP, P, KT, TPS * NTW], mdt, kind="ExternalInput"
    )
    gt_d = nc.dram_tensor("gt", [P, KT, B], mdt, kind="ExternalInput")
    eye_d = nc.dram_tensor("eye", [P, P], mybir.dt.float32, kind="ExternalInput")
    cnt_d = nc.dram_tensor("cnt", [P, NB], mybir.dt.float32, kind="ExternalOutput")
    tv_d = nc.dram_tensor("tv", [P, NB], mybir.dt.float32, kind="ExternalOutput")

    ge = mybir.AluOpType
    act = mybir.ActivationFunctionType
    with ExitStack() as ctx:
        tq = ctx.enter_context(nc.sbuf_tensor([P, KT, B], mdt))
        tg = ctx.enter_context(nc.sbuf_tensor([P, KT, B], mdt))
        teye = ctx.enter_context(nc.sbuf_tensor([P, P], mybir.dt.float32))
        trh = ctx.enter_context(nc.sbuf_tensor([P, 2, KT, TPS * NTW], mdt))
        tall = ctx.enter_context(nc.sbuf_tensor([P, NB], mybir.dt.float32))
        ntall = ctx.enter_context(nc.sbuf_tensor([P, NB], mybir.dt.float32))
        acc = ctx.enter_context(nc.sbuf_tensor([P, NB, NT], mybir.dt.float32))
        cnt = ctx.enter_context(nc.sbuf_tensor([P, NB], mybir.dt.float32))
        dscr = ctx.enter_context(nc.sbuf_tensor([P, P], mybir.dt.float32))
        cscr_v = ctx.enter_context(nc.sbuf_tensor([P, NTW], mybir.dt.bfloat16))
        cscr_a = ctx.enter_context(nc.sbuf_tensor([P, NTW], mybir.dt.bfloat16))
        psm = ctx.enter_context(nc.psum_tensor([P, NBANK, 512], mybir.dt.float32))
        pst = ctx.enter_context(nc.psum_tensor([P, 2, 512], mybir.dt.float32))

        dma_q = ctx.enter_context(nc.semaphore())
        dma_r = ctx.enter_context(nc.semaphore())
        mm_sem = ctx.enter_context(nc.semaphore())
        t_sem = ctx.enter_context(nc.semaphore())
        cv_sem = ctx.enter_context(nc.semaphore())
        ca_sem = ctx.enter_context(nc.semaphore())
        block = ctx.enter_context(nc.Block())

        # tiles 0..i-1 with i's parity among the first k done on each engine
        def _nv(j):  # DVE count-ops completed once even tile j is done
            return j // 2 + 1

        def _na(j):  # ScalarE count-ops completed once odd tile j is done
            return (j - 1) // 2 + 1

        @block.sync
        def _(sync):
            sync.dma_start(tq[:], qT_d[:]).then_inc(dma_q, 16)
            sync.dma_start(tg[:], gt_d[:]).then_inc(dma_q, 16)
            sync.dma_start(teye[:], eye_d[:]).then_inc(dma_q, 16)
            for s in range(NSUP):
                if s >= 2:
                    # PE finished all blocks of super s-2 -> buffer free
                    sync.wait_ge(mm_sem, NB + (s - 1) * TPS * NB)
                sync.dma_start(trh[:, s % 2], rhs_d[s]).then_inc(dma_r, 16)
            sync.wait_ge(t_sem, NB)
            sync.dma_start(tv_d[:], tall[:]).then_inc(dma_q, 16)
            sync.wait_ge(cv_sem, NT * NB // 2 + 1)
            sync.dma_start(cnt_d[:], cnt[:]).then_inc(dma_q, 16)

        @block.tensor
        def _(tensor):
            tensor.wait_ge(dma_q, 48)
            # t-phase: true-column scores, one [128,128] tile per block
            for b in range(NB):
                if b >= 2:
                    tensor.wait_ge(t_sem, b - 1)
                for k in range(KT):
                    mm = nc.tensor.matmul(
                        pst[:, b % 2, 0:P],
                        tq[:, k, b * P : (b + 1) * P],
                        tg[:, k, b * P : (b + 1) * P],
                        start=(k == 0),
                        stop=(k == KT - 1),
                    )
                    if k == KT - 1:
                        mm.then_inc(mm_sem, 1)
            # main loop
            for nt in range(NT):
                if nt % TPS == 0:
                    tensor.wait_ge(dma_r, (nt // TPS + 1) * 16)
                for b in range(NB):
                    i = nt * NB + b
                    if VARIANT == "split" and i >= NBANK:
                        j = i - NBANK  # previous tile in this PSUM bank
                        if j % 2 == 0:
                            tensor.wait_ge(cv_sem, _nv(j))
                        else:
                            tensor.wait_ge(ca_sem, _na(j))
                    elif VARIANT == "dveonly" and i >= NBANK:
                        tensor.wait_ge(cv_sem, i - NBANK + 1)
                    for k in range(KT):
                        mm = nc.tensor.matmul(
                            psm[:, i % NBANK, 0:NTW],
                            tq[:, k, b * P : (b + 1) * P],
                            trh[
                                :,
                                (nt // TPS) % 2,
                                k,
                                (nt % TPS) * NTW : (nt % TPS + 1) * NTW,
                            ],
                            start=(k == 0),
                            stop=(k == KT - 1),
                        )
                        if k == KT - 1:
                            mm.then_inc(mm_sem, 1)

        @block.vector
        def _(vector):
            vector.wait_ge(dma_q, 48)
            for b in range(NB):
                vector.wait_ge(mm_sem, b + 1)
                # diag extract straight out of PSUM: tall[p,b] = dscr[p,p]
                nc.vector.scalar_tensor_tensor(
                    out=dscr[:],
                    in0=pst[:, b % 2, 0:P],
                    scalar=1.0,
                    in1=teye[:],
                    op0=ge.mult,
                    op1=ge.mult,
                    accum_out=tall[:, b : b + 1],
                ).then_inc(t_sem, 1)
            nc.vector.tensor_scalar(
                ntall[:], tall[:], -1.0, None, op0=ge.mult
            ).then_inc(t_sem, 1)
            if VARIANT == "split":
                # even row blocks: strict-> count on DVE
                for nt in range(NT):
                    for b in range(0, NB, 2):
                        i = nt * NB + b
                        vector.wait_ge(mm_sem, NB + i + 1)
                        nc.vector.tensor_scalar(
                            cscr_v[:],
                            psm[:, i % NBANK, 0:NTW],
                            tall[:, b : b + 1],
                            0.0,
                            op0=ge.is_gt,
                            op1=ge.add,
                            accum_out=acc[:, b, nt : nt + 1],
                        ).then_inc(cv_sem, 1)
                vector.wait_ge(ca_sem, NT * NB // 2)
            elif VARIANT == "dveonly":
                for nt in range(NT):
                    for b in range(NB):
                        i = nt * NB + b
                        vector.wait_ge(mm_sem, NB + i + 1)
                        nc.vector.tensor_scalar(
                            cscr_v[:],
                            psm[:, i % NBANK, 0:NTW],
                            tall[:, b : b + 1],
                            0.0,
                            op0=ge.is_gt,
                            op1=ge.add,
                            accum_out=acc[:, b, nt : nt + 1],
                        ).then_inc(cv_sem, 1)
            else:  # nocnt
                vector.wait_ge(mm_sem, NB + NT * NB)
            for b in range(NB):
                red = nc.vector.tensor_reduce(
                    cnt[:, b : b + 1],
                    acc[:, b],
                    axis=mybir.AxisListType.X,
                    op=ge.add,
                )
                if b == NB - 1:
                    red.then_inc(cv_sem, 1)

        if VARIANT == "split":

            @block.scalar
            def _(scalar):
                scalar.wait_ge(t_sem, NB + 1)
                # odd row blocks: sign-sum count on ScalarE
                for nt in range(NT):
                    for b in range(1, NB, 2):
                        i = nt * NB + b
                        scalar.wait_ge(mm_sem, NB + i + 1)
                        nc.scalar.activation(
                            cscr_a[:],
                            psm[:, i % NBANK, 0:NTW],
                            act.Sign,
                            bias=ntall[:, b : b + 1],
                            scale=1.0,
                            accum_out=acc[:, b, nt : nt + 1],
                        ).then_inc(ca_sem, 1)

    return nc


def _build():
    if "nc" not in _CACHE:
        import concourse.mybir as mybir

        _CACHE["mybir"] = mybir
        _CACHE["nc"] = _gen()
    return _CACHE["nc"]


def _gen_null():
    """Same I/O signature as _gen() but near-zero work: measures the per-exec
    dispatch floor of an identical-signature bass program."""
    import concourse.bass as bass
    import concourse.mybir as mybir

    mdt = mybir.dt.float32r if F32R else mybir.dt.float32
    nc = bass.Bass()
    nc.dram_tensor("qT", [P, KT, B], mdt, kind="ExternalInput")
    nc.dram_tensor("rhsc", [NSUP, P, KT, TPS * NTW], mdt, kind="ExternalInput")
    nc.dram_tensor("gt", [P, KT, B], mdt, kind="ExternalInput")
    eye_d = nc.dram_tensor("eye", [P, P], mybir.dt.float32, kind="ExternalInput")
    cnt_d = nc.dram_tensor("cnt", [P, NB], mybir.dt.float32, kind="ExternalOutput")
    tv_d = nc.dram_tensor("tv", [P, NB], mybir.dt.float32, kind="ExternalOutput")
    with ExitStack() as ctx:
        t = ctx.enter_context(nc.sbuf_tensor([P, NB], mybir.dt.float32))
        dma_q = ctx.enter_context(nc.semaphore())
        block = ctx.enter_context(nc.Block())

        @block.sync
        def _(sync):
            sync.dma_start(t[:], eye_d[:, 0:NB]).then_inc(dma_q, 16)
            sync.wait_ge(dma_q, 16)
            sync.dma_start(cnt_d[:], t[:]).then_inc(dma_q, 16)
            sync.dma_start(tv_d[:], t[:]).then_inc(dma_q, 16)

    return nc


def _make_fn(nc, n_cores):
    """Build a jitted runner for the bass program."""
    import jax
    from jax.sharding import Mesh, PartitionSpec

    try:
        from jax.experimental.shard_map import shard_map
    except ImportError:  # newer jax
        from jax.shard_map import shard_map

    import concourse.mybir as mybir
    from concourse import bass2jax

    bass2jax.install_neuronx_cc_hook()
    partition_name = nc.partition_id_tensor.name if nc.partition_id_tensor else None
    in_names, out_names, out_avals, zero_outs = [], [], [], []
    for alloc in nc.m.functions[0].allocations:
        if not isinstance(alloc, mybir.MemoryLocationSet):
            continue
        name = alloc.memorylocations[0].name
        if alloc.kind == "ExternalInput":
            if name != partition_name:
                in_names.append(name)
        elif alloc.kind == "ExternalOutput":
            out_names.append(name)
            shape = tuple(alloc.tensor_shape)
            dtype = mybir.dt.np(alloc.dtype)
            out_avals.append(jax.core.ShapedArray(shape, dtype))
            zero_outs.append(np.zeros(shape, dtype))
    n_params = len(in_names)
    names_all = in_names + out_names + ([partition_name] if partition_name else [])

    def _body(*args):
        operands = list(args)
        if partition_name:
            operands.append(bass2jax.partition_id_tensor())
        outs = bass2jax._bass_exec_p.bind(
            *operands,
            out_avals=tuple(out_avals),
            in_names=tuple(names_all),
            out_names=tuple(out_names),
            lowering_input_output_aliases=(),
            sim_require_finite=True,
            sim_require_nnan=True,
            nc=nc,
        )
        return tuple(outs)

    devices = jax.devices()[:n_cores]
    mesh = Mesh(np.asarray(devices), ("core",))
    in_specs = (PartitionSpec("core"),) * (n_params + len(out_names))
    out_specs = (PartitionSpec("core"),) * len(out_names)
    fn = jax.jit(
        shard_map(
            _body, mesh=mesh, in_specs=in_specs, out_specs=out_specs, check_rep=False
        ),
        keep_unused=True,
    )
    return fn, in_names, out_names, out_avals, zero_outs


def _run_pjrt(nc, in_maps, n_cores, reps=0):
    """Run nc via PJRT with device-resident inputs. When reps>0, also runs a
    null bass program with the identical I/O signature interleaved with the
    real kernel and reports exec time as min(real) - min(null), cancelling the
    (noisy, several-ms) axon dispatch cost measured under the same link
    conditions."""
    import time as _time

    import jax
    from jax.sharding import Mesh, NamedSharding, PartitionSpec

    fn, in_names, out_names, out_avals, zero_outs = _make_fn(nc, n_cores)
    devices = jax.devices()[:n_cores]
    mesh = Mesh(np.asarray(devices), ("core",))
    concat_in = [
        np.concatenate([np.asarray(in_maps[c][nm]) for c in range(n_cores)], axis=0)
        for nm in in_names
    ]
    concat_zeros = [
        np.zeros((n_cores * z.shape[0], *z.shape[1:]), z.dtype) for z in zero_outs
    ]
    sh = NamedSharding(mesh, PartitionSpec("core"))
    dev_in = [jax.device_put(x, sh) for x in concat_in]
    dev_zero = [jax.device_put(x, sh) for x in concat_zeros]
    # First execution warms one-time state (act-table load path corrupts a
    # slice of SBUF mid-run on exec 1 — see kernel docstring); results are
    # taken from the second, steady-state execution.
    jax.block_until_ready(fn(*dev_in, *dev_zero))
    out = fn(*dev_in, *dev_zero)
    jax.block_until_ready(out)
    exec_s = None
    if reps:
        times = []
        for _ in range(reps):
            t0 = _time.perf_counter()
            jax.block_until_ready(fn(*dev_in, *dev_zero))
            times.append(_time.perf_counter() - t0)
        exec_s = min(times)
        _CACHE["last_raw_e2e"] = min(times)
    results = [
        {
            name: np.asarray(out[i]).reshape(n_cores, *out_avals[i].shape)[c]
            for i, name in enumerate(out_names)
        }
        for c in range(n_cores)
    ]
    return results, exec_s


def _run_device(qT, rhs, gt, eye, trace=False, reps=0):
    nc = _build()
    in_maps = []
    for c in range(NCORES):
        in_maps.append(
            {
                "qT": qT,
                "rhsc": np.ascontiguousarray(
                    rhs[:, c * COLS : (c + 1) * COLS]
                    .reshape(KT, P, NSUP, TPS * NTW)
                    .transpose(2, 1, 0, 3)
                ),
                "gt": gt,
                "eye": eye,
            }
        )
    return _run_pjrt(nc, in_maps, NCORES, reps=reps)


def kernel(q, rhs, queries, filter_idx, _trace=False, _ret_exec=False, _reps=0):
    q = np.asarray(q, dtype=np.float32)
    rhs = np.asarray(rhs, dtype=np.float32)
    true_rhs = np.asarray(queries)[:, 2].astype(np.int64)
    filt = np.asarray(filter_idx).astype(np.int64)

    qT = np.ascontiguousarray(q.T.reshape(KT, P, B).transpose(1, 0, 2))
    gt = np.ascontiguousarray(rhs[:, true_rhs].reshape(KT, P, B).transpose(1, 0, 2))
    eye = np.eye(P, dtype=np.float32)

    results, exec_s = _run_device(qT, rhs, gt, eye, reps=_reps)

    counts = np.zeros(B, dtype=np.float64)
    for c in range(NCORES):
        cc = results[c]["cnt"]  # [P, NB]
        counts += cc.T.reshape(B)  # row b*128+p = cc[p, b]
    # odd row blocks hold sign-sums: sum = #gt - #lt = 2*#gt - N + #eq,
    # and the true column's sign(0) = 0 means #eq >= 1 exactly once
    odd = (np.arange(B) // P) % 2 == 1
    counts[odd] = (counts[odd] + N - 1.0) / 2.0
    t = results[0]["tv"].T.reshape(B).astype(np.float32)  # device true scores

    # host correction: dedupe filter, drop entries equal to true tail
    q64 = q.astype(np.float64)
    corr = np.zeros(B, dtype=np.float64)
    CH = 256
    for s in range(0, B, CH):
        e = s + CH
        idx = filt[s:e]  # [CH, 64]
        cols = rhs[:, idx.reshape(-1)].astype(np.float64)  # [512, CH*64]
        sc = np.einsum(
            "bd,dbf->bf", q64[s:e], cols.reshape(D, e - s, idx.shape[1])
        )  # [CH, 64]
        gtmask = sc > t[s:e, None].astype(np.float64)
        # dedupe within row + exclude true index
        srt = np.sort(idx, axis=1)
        first = np.ones_like(idx, dtype=bool)
        order = np.argsort(idx, axis=1, kind="stable")
        dup = srt[:, 1:] == srt[:, :-1]
        fsorted = np.ones_like(idx, dtype=bool)
        fsorted[:, 1:] = ~dup
        np.put_along_axis(first, order, fsorted, axis=1)
        valid = first & (idx != true_rhs[s:e, None])
        corr[s:e] = (gtmask & valid).sum(axis=1)

    ranks = 1.0 + counts - corr
    ranks = np.maximum(np.round(ranks), 1.0).astype(np.float32)
    if _ret_exec:
        return ranks, exec_s
    return ranks
